# revision 11
# baseline (speedup 1.0000x reference)
"""BiLSTM + CRF loss kernel for Trainium2 (8 NeuronCores, data-parallel over batch).

Problem: nn_BiRNN_CRF — B=64, S=512, E=768, H=256, T=9 tags.
Output: scalar -mean(log-likelihood).

Strategy (per core, Bc=8 examples, both LSTM directions interleaved):
- gate order permuted host-side to (gc, i, f, o): tanh slice / sigmoid slice contiguous
- input projection x@W_ih^T (+bias via ones-row matmul) computed chunk-wise (16
  timesteps) directly into PSUM; the recurrent matmul h@W_hh^T accumulates onto it
  in place (bank-init matmul pre-sets has_written for the whole bank)
- LSTM weights fp8e4, activations bf16 streams, cell state fp32
- layout: gates on partitions [128p, t, 8grp, Bc] so ACT/DVE use all 128 lanes
- emissions em.T = w_proj.T @ [h_f; h_b] into PSUM [9, Bc, S]
- CRF in renormalized linear space: A_t = (expM.T @ A_{t-1}) * exp(em_t + b_proj),
  renorm every 16 steps via ln/exp (factor cancels exactly in logZ)
- numerator: one-hot masked emission sum on device; start/trans/end/b_proj path
  terms computed host-side from int inputs
"""
import sys

sys.path.insert(0, "/opt/trn_rl_repo")

import numpy as np
import ml_dtypes

from concourse import bacc, mybir, tile
from concourse.bass_utils import run_bass_kernel_spmd

BF16 = ml_dtypes.bfloat16
F32 = np.float32

B, S, E, H, T = 64, 512, 768, 256, 9
N_CORES = 8
BC = B // N_CORES  # 8 examples per core
CH = 16  # timesteps per projection chunk
R_RENORM = 16
GATE_PERM = (2, 0, 1, 3)  # (i,f,gc,o) -> (gc,i,f,o)
KE = E // 128  # 6 K-chunks for input projection
KH = H // 128  # 2 K-chunks for recurrence
MG = 4 * H // 128  # 8 M-tiles of gates
DT8 = mybir.dt.float8e4
DTB = mybir.dt.bfloat16
DTF = mybir.dt.float32
FP8 = np.dtype(mybir.dt.np(DT8))


def build_nc(num_devices=N_CORES, s_steps=S, debug=False):
    """Build the SPMD program (identical on all cores)."""
    SS = s_steps
    NCH = SS // CH
    nc = bacc.Bacc("TRN2", target_bir_lowering=False, debug=False, num_devices=num_devices)

    dp = lambda name, shape, dt: nc.declare_dram_parameter(name, list(shape), dt, isOutput=False)
    # inputs (per core shard)
    xT_d = dp("xT", [128, KE, SS, BC], DTB)  # x transposed [p, k, t, b]
    wih_d = {d: dp(f"wih_{d}", [128, KE, MG, 128], DT8) for d in "fb"}
    whh_d = {d: dp(f"whh_{d}", [128, MG, KH, 128], DT8) for d in "fb"}
    bias_d = {d: dp(f"bias_{d}", [1, MG, 128], DT8) for d in "fb"}
    wproj_d = dp("wproj", [128, 4, T], DTB)
    expM_d = dp("expM", [T, T], DTF)
    expst_d = dp("expst", [T, 1], DTF)
    expend_d = dp("expend", [T, 1], DTF)
    bproj_d = dp("bproj", [T, 1], DTF)
    oh_d = dp("oh", [T, BC, SS], DTB)
    out_d = nc.declare_dram_parameter("out_nm", [2, BC], DTF, isOutput=True)
    if debug:
        hf_dbg = nc.declare_dram_parameter("h_f_dbg", [128, KH, BC, SS], DTB, isOutput=True)
        hb_dbg = nc.declare_dram_parameter("h_b_dbg", [128, KH, BC, SS], DTB, isOutput=True)
        em_dbg = nc.declare_dram_parameter("em_dbg", [T, BC, SS], DTF, isOutput=True)

    with tile.TileContext(nc) as tc:
        with (
            tc.tile_pool(name="const", bufs=1) as cpool,
            tc.tile_pool(name="xchunks", bufs=4) as xpool,
            tc.tile_pool(name="cell", bufs=3) as spool,
            tc.tile_pool(name="crf", bufs=3) as crfpool,
        ):
            # ---- persistent SBUF tiles
            wih = {d: cpool.tile([128, KE, MG, 128], DT8, tag=f"wih{d}", name=f"wih{d}") for d in "fb"}
            whh = {d: cpool.tile([128, MG, KH, 128], DT8, tag=f"whh{d}", name=f"whh{d}") for d in "fb"}
            bias = {d: cpool.tile([1, MG, 128], DT8, tag=f"bias{d}", name=f"bias{d}") for d in "fb"}
            wproj = cpool.tile([128, 4, T], DTB, tag="wproj", name="wproj")
            expM = cpool.tile([T, T], DTF, tag="expM", name="expM")
            expst = cpool.tile([T, 1], DTF, tag="expst", name="expst")
            expend = cpool.tile([T, 1], DTF, tag="expend", name="expend")
            bproj = cpool.tile([T, 1], DTF, tag="bproj", name="bproj")
            oh = cpool.tile([T, BC, SS], DTB, tag="oh", name="oh")
            hst = {d: cpool.tile([128, KH, BC, SS], DTB, tag=f"hst{d}", name=f"hst{d}") for d in "fb"}
            ct = {d: cpool.tile([128, KH, BC], DTF, tag=f"c{d}", name=f"c{d}") for d in "fb"}
            ones_row = cpool.tile([1, 512], DTB, tag="ones_row", name="ones_row")
            zrow = cpool.tile([1, 128], DT8, tag="zrow", name="zrow")
            ones9 = cpool.tile([T, 1], DTF, tag="ones9", name="ones9")
            ones19 = cpool.tile([1, T], DTF, tag="ones19", name="ones19")
            E_sb = cpool.tile([T, BC, SS], DTF, tag="E_sb", name="E_sb")
            lacc = cpool.tile([1, BC], DTF, tag="lacc", name="lacc")
            numemit = cpool.tile([1, BC], DTF, tag="numemit", name="numemit")
            logz = cpool.tile([1, BC], DTF, tag="logz", name="logz")

            for d in "fb":
                nc.sync.dma_start(wih[d][:], wih_d[d][:])
                nc.sync.dma_start(bias[d][:], bias_d[d][:])
                nc.sync.dma_start(whh[d][:], whh_d[d][:])
            nc.vector.memset(ones_row[:], 1.0)
            nc.vector.memset(zrow[:], 0.0)
            nc.vector.memset(ones9[:], 1.0)
            nc.vector.memset(ones19[:], 1.0)
            nc.vector.memset(lacc[:], 0.0)
            for d in "fb":
                nc.vector.memset(ct[d][:], 0.0)

            # ---- phase 1: projection + recurrence
            with tc.tile_pool(name="gpsum", bufs=2, space="PSUM") as gpool:
                xt = {}  # x chunk sbuf tiles per (dir, chunk parity)
                gps = {}  # psum chunk tensors

                def t0_of(d, c):
                    # first global timestep of chunk c's projection slice
                    return c * CH if d == "f" else SS - (c + 1) * CH

                def emit_chunk_dma(d, c):
                    t0 = t0_of(d, c)
                    xtile = xpool.tile([128, KE, CH, BC], DTB, tag=f"x{d}", name=f"x{d}")
                    nc.sync.dma_start(xtile[:], xT_d[:, :, t0 : t0 + CH, :])
                    xt[(d, c)] = xtile

                def proj_thunks(d, c):
                    """Projection of chunk c (dir d) as a list of emission thunks
                    (spread between recurrence steps so they fill PE idle gaps)."""
                    g = gpool.tile([128, MG, CH, BC], DTF, tag=f"g{d}", name=f"g{d}")
                    gps[(d, c)] = g
                    xtile = xt[(d, c)]
                    half = MG // 2
                    thunks = []
                    # start=True only on the first matmul touching each PSUM bank
                    # (clears has_written bank-wide; everything later accumulates)
                    for m in range(MG):
                        for k in range(KE):
                            thunks.append(lambda m=m, k=k: nc.tensor.matmul(
                                g[:, m, :, :],
                                wih[d][:, k, m, :],
                                xtile[:, k, :, :],
                                start=(k == 0 and m % half == 0), stop=False,
                                skip_group_check=True,
                            ))
                        thunks.append(lambda m=m: nc.tensor.matmul(
                            g[:, m, :, :],
                            bias[d][:, m, :],
                            ones_row[:, 0 : CH * BC],
                            start=False, stop=False, skip_group_check=True,
                        ))
                    return thunks

                def glob_t(d, c, j):
                    return c * CH + j if d == "f" else SS - 1 - c * CH - j

                def step_mms(d, c, j):
                    t = glob_t(d, c, j)
                    jj = j if d == "f" else CH - 1 - j
                    g = gps[(d, c)]
                    if c == 0 and j == 0:
                        return
                    tprev = t + 1 if d == "b" else t - 1
                    # k-outer: all k=0 matmuls only need h grp0 (written first)
                    for k in range(KH):
                        for m in range(MG):
                            nc.tensor.matmul(
                                g[:, m, jj, :],
                                whh[d][:, m, k, :],
                                hst[d][:, k, :, tprev],
                                start=False,
                                stop=(m == MG - 1 and k == KH - 1),
                                skip_group_check=True,
                            )

                def step_act1(d, c, j):
                    jj = j if d == "f" else CH - 1 - j
                    g = gps[(d, c)]
                    sg = spool.tile([128, 6, BC], DTF, tag=f"sg{d}", name=f"sg{d}")
                    tg = spool.tile([128, 2, BC], DTF, tag=f"tg{d}", name=f"tg{d}")
                    nc.scalar.activation(sg[:], g[:, 2:8, jj, :], mybir.ActivationFunctionType.Sigmoid)
                    nc.scalar.activation(tg[:], g[:, 0:2, jj, :], mybir.ActivationFunctionType.Tanh)
                    return sg, tg

                def step_dve1(d, sg, tg):
                    # cell update (i=sg[0:2], f=sg[2:4], o=sg[4:6])
                    v = spool.tile([128, 2, BC], DTF, tag=f"v{d}", name=f"v{d}")
                    u = spool.tile([128, 2, BC], DTF, tag=f"u{d}", name=f"u{d}")
                    nc.vector.tensor_tensor(v[:], sg[:, 2:4, :], ct[d][:], mybir.AluOpType.mult)
                    nc.vector.tensor_tensor(u[:], sg[:, 0:2, :], tg[:], mybir.AluOpType.mult)
                    nc.vector.tensor_tensor(ct[d][:], u[:], v[:], mybir.AluOpType.add)

                def step_act2(d):
                    th = spool.tile([128, 2, BC], DTF, tag=f"th{d}", name=f"th{d}")
                    nc.scalar.activation(th[:], ct[d][:], mybir.ActivationFunctionType.Tanh)
                    return th

                def step_dve2(d, c, j, sg, th):
                    # split by h-group so grp0 lands first (k=0 matmuls unblock)
                    t = glob_t(d, c, j)
                    for k in range(KH):
                        nc.vector.tensor_tensor(
                            hst[d][:, k, :, t], sg[:, 4 + k, :], th[:, k, :],
                            mybir.AluOpType.mult,
                        )

                def emit_step(d, c, j):
                    # full per-direction sequence: keeps each engine's in-order
                    # queue free of cross-direction stalls
                    step_mms(d, c, j)
                    sg, tg = step_act1(d, c, j)
                    step_dve1(d, sg, tg)
                    th = step_act2(d)
                    step_dve2(d, c, j, sg, th)

                # prologue: chunk 0+1 for both dirs (x DMAs prefetch 2 chunks deep)
                for d in "fb":
                    emit_chunk_dma(d, 0)
                for d in "fb":
                    if NCH > 1:
                        emit_chunk_dma(d, 1)
                    for th_ in proj_thunks(d, 0):
                        th_()
                nc.sync.dma_start(wproj[:], wproj_d[:])
                nc.sync.dma_start(expM[:], expM_d[:])
                nc.sync.dma_start(expst[:], expst_d[:])
                nc.sync.dma_start(expend[:], expend_d[:])
                nc.sync.dma_start(bproj[:], bproj_d[:])
                nc.sync.dma_start(oh[:], oh_d[:])
                for c in range(NCH):
                    thunks = []
                    if c + 1 < NCH:
                        thunks = proj_thunks("f", c + 1) + proj_thunks("b", c + 1)
                    # spread proj over slots 2..CH-1: slot-0/1 thunks would reach the
                    # PE queue head before the psum buffer / x DMA are ready and
                    # stall the in-order queue
                    lo = 2 if CH > 4 else 0
                    per = (len(thunks) + (CH - lo) - 1) // (CH - lo) if thunks else 0
                    for j in range(CH):
                        emit_step("f", c, j)
                        emit_step("b", c, j)
                        if j == 0 and c + 2 < NCH:
                            for d in "fb":
                                emit_chunk_dma(d, c + 2)
                        if thunks and j >= lo:
                            for th_ in thunks[(j - lo) * per : (j - lo + 1) * per]:
                                th_()

            if debug:
                for d, dbg in (("f", hf_dbg), ("b", hb_dbg)):
                    nc.sync.dma_start(dbg[:], hst[d][:])

            # ---- phase 2: emissions + numerator + CRF
            with tc.tile_pool(name="empsum", bufs=1, space="PSUM") as empool:
                em = empool.tile([T, BC, SS], DTF, tag="em", name="em")
                for b in range(BC):
                    for k in range(4):
                        d = "f" if k < 2 else "b"
                        nc.tensor.matmul(
                            em[:, b, :],
                            wproj[:, k, :],
                            hst[d][:, k % 2, b, :],
                            start=(k == 0), stop=(k == 3),
                        )
                if debug:
                    emdbg_sb = crfpool.tile([T, BC, SS], DTF, tag="emdbg", name="emdbg")
                    nc.vector.tensor_copy(emdbg_sb[:], em[:])
                    nc.sync.dma_start(em_dbg[:], emdbg_sb[:])
                # E = exp(em + b_proj)
                nc.scalar.activation(E_sb[:], em[:], mybir.ActivationFunctionType.Exp, bias=bproj[:])
                # numerator: masked emission sum
                msk = crfpool.tile([T, BC, SS], DTF, tag="msk", name="msk")
                nc.vector.tensor_tensor(msk[:], em[:], oh[:], mybir.AluOpType.mult)
                red = crfpool.tile([T, BC], DTF, tag="red", name="red")
                nc.vector.tensor_reduce(red[:], msk[:], mybir.AxisListType.X, mybir.AluOpType.add)

            NREN = (SS - 1) // R_RENORM
            with tc.tile_pool(name="crfpsum", bufs=2, space="PSUM") as apool:
                ne_ps = apool.tile([1, BC], DTF, tag="s", name="s")
                nc.tensor.matmul(ne_ps[:], ones9[:], red[:], start=True, stop=True)
                nc.vector.tensor_copy(numemit[:], ne_ps[:])
                sstore = crfpool.tile([1, BC, max(NREN, 1)], DTF, tag="sstore",
                                      name="sstore", bufs=1)

                # CRF linear-space recursion (no ACT in the loop: renorm via
                # DVE reciprocal, logs of the saved scales taken once at the end)
                A = crfpool.tile([T, BC], DTF, tag="A", name="A")
                nc.vector.tensor_scalar_mul(A[:], E_sb[:, :, 0], expst[:])
                ridx = 0
                for t in range(1, SS):
                    A_ps = apool.tile([T, BC], DTF, tag="Aps", name="Aps")
                    nc.tensor.matmul(A_ps[:], expM[:], A[:], start=True, stop=True)
                    A = crfpool.tile([T, BC], DTF, tag="A", name="A")
                    nc.vector.tensor_tensor(A[:], A_ps[:], E_sb[:, :, t], mybir.AluOpType.mult)
                    if t % R_RENORM == 0:
                        s_ps = apool.tile([1, BC], DTF, tag="s", name="s")
                        nc.tensor.matmul(s_ps[:], ones9[:], A[:], start=True, stop=True)
                        nc.vector.tensor_copy(sstore[:, :, ridx], s_ps[:])
                        rinv = crfpool.tile([1, BC], DTF, tag="rinv", name="rinv")
                        nc.vector.reciprocal(rinv[:], s_ps[:])
                        bc_ps = apool.tile([T, BC], DTF, tag="Aps", name="Aps")
                        nc.tensor.matmul(bc_ps[:], ones19[:], rinv[:], start=True, stop=True)
                        An = crfpool.tile([T, BC], DTF, tag="A", name="A")
                        nc.vector.tensor_tensor(An[:], A[:], bc_ps[:], mybir.AluOpType.mult)
                        A = An
                        ridx += 1
                # finalize: logZ = ln(sum_j A*exp(end)) + sum_k ln(s_k)
                Afin = crfpool.tile([T, BC], DTF, tag="A", name="A")
                nc.vector.tensor_scalar_mul(Afin[:], A[:], expend[:])
                zb_ps = apool.tile([1, BC], DTF, tag="s", name="s")
                nc.tensor.matmul(zb_ps[:], ones9[:], Afin[:], start=True, stop=True)
                lz = crfpool.tile([1, BC], DTF, tag="ls", name="ls")
                nc.scalar.activation(lz[:], zb_ps[:], mybir.ActivationFunctionType.Ln)
                if ridx > 0:
                    lnS = crfpool.tile([1, BC, NREN], DTF, tag="lnS", name="lnS")
                    nc.scalar.activation(lnS[:], sstore[:, :, 0:ridx],
                                         mybir.ActivationFunctionType.Ln)
                    nc.vector.tensor_reduce(lacc[:], lnS[:], mybir.AxisListType.X,
                                            mybir.AluOpType.add)
                nc.vector.tensor_tensor(logz[:], lz[:], lacc[:], mybir.AluOpType.add)

            nc.sync.dma_start(out_d[0:1, :], numemit[:])
            nc.sync.dma_start(out_d[1:2, :], logz[:])

    nc.compile()
    return nc


# ---------------- host-side preparation ----------------

def _permute_gates(w):
    parts = np.split(np.asarray(w), 4, axis=0)
    return np.concatenate([parts[k] for k in GATE_PERM], axis=0)


def prep_shared(w_ih_f, w_hh_f, b_f, w_ih_b, w_hh_b, b_b, w_proj,
                start_trans, end_trans, transitions):
    out = {}
    for d, (wi, wh, bb) in (("f", (w_ih_f, w_hh_f, b_f)), ("b", (w_ih_b, w_hh_b, b_b))):
        wiP = _permute_gates(wi)  # [4H, E]
        whP = _permute_gates(wh)  # [4H, H]
        bP = _permute_gates(np.asarray(bb)[:, None])[:, 0]
        out[f"wih_{d}"] = np.ascontiguousarray(
            wiP.reshape(MG, 128, KE, 128).transpose(3, 2, 0, 1)
        ).astype(FP8)
        out[f"whh_{d}"] = np.ascontiguousarray(
            whP.reshape(MG, 128, KH, 128).transpose(3, 0, 2, 1)
        ).astype(FP8)
        out[f"bias_{d}"] = bP.reshape(1, MG, 128).astype(FP8)
    out["wproj"] = np.ascontiguousarray(
        np.asarray(w_proj).reshape(T, 4, 128).transpose(2, 1, 0)
    ).astype(BF16)
    out["expM"] = np.exp(np.asarray(transitions, F32))
    out["expst"] = np.exp(np.asarray(start_trans, F32))[:, None]
    out["expend"] = np.exp(np.asarray(end_trans, F32))[:, None]
    return out


def prep_core(emb_shard, tags_shard, b_proj):
    xT = np.ascontiguousarray(
        np.asarray(emb_shard).reshape(BC, S, KE, 128).transpose(3, 2, 1, 0)
    ).astype(BF16)
    oh = np.zeros((T, BC, S), BF16)
    bt = np.arange(BC)[:, None], np.arange(S)[None, :]
    ohf = np.zeros((BC, S, T), np.float32)
    np.put_along_axis(ohf, np.asarray(tags_shard)[..., None], 1.0, axis=-1)
    oh = np.ascontiguousarray(ohf.transpose(2, 0, 1)).astype(BF16)
    return {"xT": xT, "oh": oh, "bproj": np.asarray(b_proj, F32)[:, None]}


def host_path_const(tags, start, end, trans, b_proj):
    tags = np.asarray(tags)
    num = np.asarray(start, F32)[tags[:, 0]]
    num = num + np.asarray(trans, F32)[tags[:, :-1], tags[:, 1:]].sum(axis=1)
    num = num + np.asarray(end, F32)[tags[:, -1]]
    num = num + np.asarray(b_proj, F32)[tags].sum(axis=1)
    return num


_NC_CACHE = {}


def _get_nc(num_devices=N_CORES, s_steps=S, debug=False):
    key = (num_devices, s_steps, debug)
    if key not in _NC_CACHE:
        _NC_CACHE[key] = build_nc(num_devices, s_steps, debug)
    return _NC_CACHE[key]


def kernel(embedding, target_tag, attention_masks, w_ih_f, w_hh_f, b_f,
           w_ih_b, w_hh_b, b_b, w_proj, b_proj, start_trans, end_trans,
           transitions, _debug=False, _trace=False, _tmpdir=None):
    embedding = np.asarray(embedding)
    target_tag = np.asarray(target_tag, np.int32)
    shared = prep_shared(w_ih_f, w_hh_f, b_f, w_ih_b, w_hh_b, b_b, w_proj,
                         start_trans, end_trans, transitions)
    nc = _get_nc(N_CORES, S, _debug)
    in_maps = []
    num_hosts = []
    for i in range(N_CORES):
        sl = slice(i * BC, (i + 1) * BC)
        m = dict(shared)
        m.update(prep_core(embedding[sl], target_tag[sl], b_proj))
        in_maps.append(m)
        num_hosts.append(host_path_const(target_tag[sl], start_trans, end_trans,
                                         transitions, b_proj))
    kw = {}
    if _trace:
        kw = {"trace": True, "tmpdir": _tmpdir}
    res = run_bass_kernel_spmd(nc, in_maps, list(range(N_CORES)), **kw)
    llh = np.zeros((B,), F32)
    for i in range(N_CORES):
        o = res.results[i]["out_nm"]
        llh[i * BC : (i + 1) * BC] = num_hosts[i] + o[0] - o[1]
    out = F32(-llh.mean())
    if _debug or _trace:
        kernel.last_results = res
    return out


# revision 12
# speedup vs baseline: 1.1081x; 1.1081x over previous
"""BiLSTM + CRF loss kernel for Trainium2 (8 NeuronCores, data-parallel over batch).

Problem: nn_BiRNN_CRF — B=64, S=512, E=768, H=256, T=9 tags.
Output: scalar -mean(log-likelihood).

Strategy (per core, Bc=8 examples, both LSTM directions interleaved):
- gate order permuted host-side to (gc, i, f, o): tanh slice / sigmoid slice contiguous
- input projection x@W_ih^T (+bias via ones-row matmul) computed chunk-wise (16
  timesteps) directly into PSUM; the recurrent matmul h@W_hh^T accumulates onto it
  in place (bank-init matmul pre-sets has_written for the whole bank)
- LSTM weights fp8e4, activations bf16 streams, cell state fp32
- layout: gates on partitions [128p, t, 8grp, Bc] so ACT/DVE use all 128 lanes
- emissions em.T = w_proj.T @ [h_f; h_b] into PSUM [9, Bc, S]
- CRF in renormalized linear space: A_t = (expM.T @ A_{t-1}) * exp(em_t + b_proj),
  renorm every 16 steps via ln/exp (factor cancels exactly in logZ)
- numerator: one-hot masked emission sum on device; start/trans/end/b_proj path
  terms computed host-side from int inputs
"""
import sys

sys.path.insert(0, "/opt/trn_rl_repo")

import numpy as np
import ml_dtypes

from concourse import bacc, mybir, tile
from concourse.bass_utils import run_bass_kernel_spmd

BF16 = ml_dtypes.bfloat16
F32 = np.float32

B, S, E, H, T = 64, 512, 768, 256, 9
N_CORES = 8
BC = B // N_CORES  # 8 examples per core
CH = 16  # timesteps per projection chunk
R_RENORM = 16
GATE_PERM = (2, 0, 1, 3)  # (i,f,gc,o) -> (gc,i,f,o)
KE = E // 128  # 6 K-chunks for input projection
KH = H // 128  # 2 K-chunks for recurrence
MG = 4 * H // 128  # 8 M-tiles of gates
DT8 = mybir.dt.float8e4
DTB = mybir.dt.bfloat16
DTF = mybir.dt.float32
FP8 = np.dtype(mybir.dt.np(DT8))


def build_nc(num_devices=N_CORES, s_steps=S, debug=False):
    """Build the SPMD program (identical on all cores)."""
    SS = s_steps
    NCH = SS // CH
    nc = bacc.Bacc("TRN2", target_bir_lowering=False, debug=False, num_devices=num_devices)

    dp = lambda name, shape, dt: nc.declare_dram_parameter(name, list(shape), dt, isOutput=False)
    # inputs (per core shard)
    xT_d = dp("xT", [128, KE, SS, BC], DTB)  # x transposed [p, k, t, b]
    wih_d = {d: dp(f"wih_{d}", [128, KE, MG, 128], DT8) for d in "fb"}
    whh_d = {d: dp(f"whh_{d}", [128, MG, KH, 128], DT8) for d in "fb"}
    bias_d = {d: dp(f"bias_{d}", [1, MG, 128], DT8) for d in "fb"}
    wproj_d = dp("wproj", [128, 4, T], DTB)
    expM_d = dp("expM", [T, T], DTF)
    expst_d = dp("expst", [T, 1], DTF)
    expend_d = dp("expend", [T, 1], DTF)
    bproj_d = dp("bproj", [T, 1], DTF)
    oh_d = dp("oh", [T, BC, SS], DTB)
    out_d = nc.declare_dram_parameter("out_nm", [2, BC], DTF, isOutput=True)
    if debug:
        hf_dbg = nc.declare_dram_parameter("h_f_dbg", [128, KH, BC, SS], DTB, isOutput=True)
        hb_dbg = nc.declare_dram_parameter("h_b_dbg", [128, KH, BC, SS], DTB, isOutput=True)
        em_dbg = nc.declare_dram_parameter("em_dbg", [T, BC, SS], DTF, isOutput=True)

    with tile.TileContext(nc) as tc:
        with (
            tc.tile_pool(name="const", bufs=1) as cpool,
            tc.tile_pool(name="xchunks", bufs=4) as xpool,
            tc.tile_pool(name="cell", bufs=3) as spool,
            tc.tile_pool(name="crf", bufs=3) as crfpool,
        ):
            # ---- persistent SBUF tiles
            wih = {d: cpool.tile([128, KE, MG, 128], DT8, tag=f"wih{d}", name=f"wih{d}") for d in "fb"}
            whh = {d: cpool.tile([128, MG, KH, 128], DT8, tag=f"whh{d}", name=f"whh{d}") for d in "fb"}
            bias = {d: cpool.tile([1, MG, 128], DT8, tag=f"bias{d}", name=f"bias{d}") for d in "fb"}
            wproj = cpool.tile([128, 4, T], DTB, tag="wproj", name="wproj")
            expM = cpool.tile([T, T], DTF, tag="expM", name="expM")
            expst = cpool.tile([T, 1], DTF, tag="expst", name="expst")
            expend = cpool.tile([T, 1], DTF, tag="expend", name="expend")
            bproj = cpool.tile([T, 1], DTF, tag="bproj", name="bproj")
            oh = cpool.tile([T, BC, SS], DTB, tag="oh", name="oh")
            hst = {d: cpool.tile([128, KH, BC, SS], DTB, tag=f"hst{d}", name=f"hst{d}") for d in "fb"}
            ct = {d: cpool.tile([128, KH, BC], DTF, tag=f"c{d}", name=f"c{d}") for d in "fb"}
            ones_row = cpool.tile([1, 512], DTB, tag="ones_row", name="ones_row")
            zrow = cpool.tile([1, 128], DT8, tag="zrow", name="zrow")
            ones9 = cpool.tile([T, 1], DTF, tag="ones9", name="ones9")
            ones19 = cpool.tile([1, T], DTF, tag="ones19", name="ones19")
            E_sb = cpool.tile([T, BC, SS], DTF, tag="E_sb", name="E_sb")
            lacc = cpool.tile([1, BC], DTF, tag="lacc", name="lacc")
            numemit = cpool.tile([1, BC], DTF, tag="numemit", name="numemit")
            logz = cpool.tile([1, BC], DTF, tag="logz", name="logz")

            for d in "fb":
                nc.sync.dma_start(wih[d][:], wih_d[d][:])
                nc.sync.dma_start(bias[d][:], bias_d[d][:])
                nc.sync.dma_start(whh[d][:], whh_d[d][:])
            nc.vector.memset(ones_row[:], 1.0)
            nc.vector.memset(zrow[:], 0.0)
            nc.vector.memset(ones9[:], 1.0)
            nc.vector.memset(ones19[:], 1.0)
            nc.vector.memset(lacc[:], 0.0)
            for d in "fb":
                nc.vector.memset(ct[d][:], 0.0)

            # ---- phase 1: projection + recurrence
            with tc.tile_pool(name="gpsum", bufs=2, space="PSUM") as gpool:
                xt = {}  # x chunk sbuf tiles per (dir, chunk parity)
                gps = {}  # psum chunk tensors

                def t0_of(d, c):
                    # first global timestep of chunk c's projection slice
                    return c * CH if d == "f" else SS - (c + 1) * CH

                def emit_chunk_dma(d, c):
                    t0 = t0_of(d, c)
                    xtile = xpool.tile([128, KE, CH, BC], DTB, tag=f"x{d}", name=f"x{d}")
                    nc.sync.dma_start(xtile[:], xT_d[:, :, t0 : t0 + CH, :])
                    xt[(d, c)] = xtile

                def proj_thunks(d, c):
                    """Projection of chunk c (dir d) as a list of emission thunks
                    (spread between recurrence steps so they fill PE idle gaps)."""
                    g = gpool.tile([128, MG, CH, BC], DTF, tag=f"g{d}", name=f"g{d}")
                    gps[(d, c)] = g
                    xtile = xt[(d, c)]
                    half = MG // 2
                    thunks = []
                    # k-outer so consecutive matmuls hit different PSUM regions
                    # (same-dst accumulation back-to-back breaks PE pipelining).
                    # start=True only on the first matmul touching each PSUM bank
                    # (clears has_written bank-wide; everything later accumulates)
                    for k in range(KE):
                        for m in range(MG):
                            thunks.append(lambda m=m, k=k: nc.tensor.matmul(
                                g[:, m, :, :],
                                wih[d][:, k, m, :],
                                xtile[:, k, :, :],
                                start=(k == 0 and m % half == 0), stop=False,
                                skip_group_check=True,
                            ))
                    for m in range(MG):
                        thunks.append(lambda m=m: nc.tensor.matmul(
                            g[:, m, :, :],
                            bias[d][:, m, :],
                            ones_row[:, 0 : CH * BC],
                            start=False, stop=False, skip_group_check=True,
                        ))
                    return thunks

                def glob_t(d, c, j):
                    return c * CH + j if d == "f" else SS - 1 - c * CH - j

                def step_mms(d, c, j):
                    t = glob_t(d, c, j)
                    jj = j if d == "f" else CH - 1 - j
                    g = gps[(d, c)]
                    if c == 0 and j == 0:
                        return
                    tprev = t + 1 if d == "b" else t - 1
                    # k-outer: all k=0 matmuls only need h grp0 (written first)
                    for k in range(KH):
                        for m in range(MG):
                            nc.tensor.matmul(
                                g[:, m, jj, :],
                                whh[d][:, m, k, :],
                                hst[d][:, k, :, tprev],
                                start=False,
                                stop=(m == MG - 1 and k == KH - 1),
                                skip_group_check=True,
                            )

                def step_act1(d, c, j):
                    jj = j if d == "f" else CH - 1 - j
                    g = gps[(d, c)]
                    sg = spool.tile([128, 6, BC], DTF, tag=f"sg{d}", name=f"sg{d}")
                    tg = spool.tile([128, 2, BC], DTF, tag=f"tg{d}", name=f"tg{d}")
                    nc.scalar.activation(sg[:], g[:, 2:8, jj, :], mybir.ActivationFunctionType.Sigmoid)
                    nc.scalar.activation(tg[:], g[:, 0:2, jj, :], mybir.ActivationFunctionType.Tanh)
                    return sg, tg

                def step_dve1(d, sg, tg):
                    # cell update (i=sg[0:2], f=sg[2:4], o=sg[4:6])
                    v = spool.tile([128, 2, BC], DTF, tag=f"v{d}", name=f"v{d}")
                    u = spool.tile([128, 2, BC], DTF, tag=f"u{d}", name=f"u{d}")
                    nc.vector.tensor_tensor(v[:], sg[:, 2:4, :], ct[d][:], mybir.AluOpType.mult)
                    nc.vector.tensor_tensor(u[:], sg[:, 0:2, :], tg[:], mybir.AluOpType.mult)
                    nc.vector.tensor_tensor(ct[d][:], u[:], v[:], mybir.AluOpType.add)

                def step_act2(d):
                    th = spool.tile([128, 2, BC], DTF, tag=f"th{d}", name=f"th{d}")
                    nc.scalar.activation(th[:], ct[d][:], mybir.ActivationFunctionType.Tanh)
                    return th

                def step_dve2(d, c, j, sg, th):
                    # split by h-group so grp0 lands first (k=0 matmuls unblock)
                    t = glob_t(d, c, j)
                    for k in range(KH):
                        nc.vector.tensor_tensor(
                            hst[d][:, k, :, t], sg[:, 4 + k, :], th[:, k, :],
                            mybir.AluOpType.mult,
                        )

                def emit_step(d, c, j):
                    # full per-direction sequence: keeps each engine's in-order
                    # queue free of cross-direction stalls
                    step_mms(d, c, j)
                    sg, tg = step_act1(d, c, j)
                    step_dve1(d, sg, tg)
                    th = step_act2(d)
                    step_dve2(d, c, j, sg, th)

                # prologue: chunk 0+1 for both dirs (x DMAs prefetch 2 chunks deep)
                for d in "fb":
                    emit_chunk_dma(d, 0)
                for d in "fb":
                    if NCH > 1:
                        emit_chunk_dma(d, 1)
                    for th_ in proj_thunks(d, 0):
                        th_()
                nc.sync.dma_start(wproj[:], wproj_d[:])
                nc.sync.dma_start(expM[:], expM_d[:])
                nc.sync.dma_start(expst[:], expst_d[:])
                nc.sync.dma_start(expend[:], expend_d[:])
                nc.sync.dma_start(bproj[:], bproj_d[:])
                nc.sync.dma_start(oh[:], oh_d[:])
                for c in range(NCH):
                    thunks = []
                    if c + 1 < NCH:
                        thunks = proj_thunks("f", c + 1) + proj_thunks("b", c + 1)
                    # spread proj over slots 2..CH-1: slot-0/1 thunks would reach the
                    # PE queue head before the psum buffer / x DMA are ready and
                    # stall the in-order queue
                    lo = 2 if CH > 4 else 0
                    per = (len(thunks) + (CH - lo) - 1) // (CH - lo) if thunks else 0
                    for j in range(CH):
                        emit_step("f", c, j)
                        emit_step("b", c, j)
                        if j == 0 and c + 2 < NCH:
                            for d in "fb":
                                emit_chunk_dma(d, c + 2)
                        if thunks and j >= lo:
                            for th_ in thunks[(j - lo) * per : (j - lo + 1) * per]:
                                th_()

            if debug:
                for d, dbg in (("f", hf_dbg), ("b", hb_dbg)):
                    nc.sync.dma_start(dbg[:], hst[d][:])

            # ---- phase 2: emissions + numerator + CRF
            with tc.tile_pool(name="empsum", bufs=1, space="PSUM") as empool:
                em = empool.tile([T, BC, SS], DTF, tag="em", name="em")
                for b in range(BC):
                    for k in range(4):
                        d = "f" if k < 2 else "b"
                        nc.tensor.matmul(
                            em[:, b, :],
                            wproj[:, k, :],
                            hst[d][:, k % 2, b, :],
                            start=(k == 0), stop=(k == 3),
                        )
                if debug:
                    emdbg_sb = crfpool.tile([T, BC, SS], DTF, tag="emdbg", name="emdbg")
                    nc.vector.tensor_copy(emdbg_sb[:], em[:])
                    nc.sync.dma_start(em_dbg[:], emdbg_sb[:])
                # E = exp(em + b_proj)
                nc.scalar.activation(E_sb[:], em[:], mybir.ActivationFunctionType.Exp, bias=bproj[:])
                # numerator: masked emission sum
                msk = crfpool.tile([T, BC, SS], DTF, tag="msk", name="msk")
                nc.vector.tensor_tensor(msk[:], em[:], oh[:], mybir.AluOpType.mult)
                red = crfpool.tile([T, BC], DTF, tag="red", name="red")
                nc.vector.tensor_reduce(red[:], msk[:], mybir.AxisListType.X, mybir.AluOpType.add)

            NREN = (SS - 1) // R_RENORM
            with tc.tile_pool(name="crfpsum", bufs=2, space="PSUM") as apool:
                ne_ps = apool.tile([1, BC], DTF, tag="s", name="s")
                nc.tensor.matmul(ne_ps[:], ones9[:], red[:], start=True, stop=True)
                nc.vector.tensor_copy(numemit[:], ne_ps[:])
                sstore = crfpool.tile([1, BC, max(NREN, 1)], DTF, tag="sstore",
                                      name="sstore", bufs=1)

                # CRF linear-space recursion (no ACT in the loop: renorm via
                # DVE reciprocal, logs of the saved scales taken once at the end)
                A = crfpool.tile([T, BC], DTF, tag="A", name="A")
                nc.vector.tensor_scalar_mul(A[:], E_sb[:, :, 0], expst[:])
                ridx = 0
                for t in range(1, SS):
                    A_ps = apool.tile([T, BC], DTF, tag="Aps", name="Aps")
                    nc.tensor.matmul(A_ps[:], expM[:], A[:], start=True, stop=True)
                    A = crfpool.tile([T, BC], DTF, tag="A", name="A")
                    nc.vector.tensor_tensor(A[:], A_ps[:], E_sb[:, :, t], mybir.AluOpType.mult)
                    if t % R_RENORM == 0:
                        s_ps = apool.tile([1, BC], DTF, tag="s", name="s")
                        nc.tensor.matmul(s_ps[:], ones9[:], A[:], start=True, stop=True)
                        nc.vector.tensor_copy(sstore[:, :, ridx], s_ps[:])
                        rinv = crfpool.tile([1, BC], DTF, tag="rinv", name="rinv")
                        nc.vector.reciprocal(rinv[:], s_ps[:])
                        bc_ps = apool.tile([T, BC], DTF, tag="Aps", name="Aps")
                        nc.tensor.matmul(bc_ps[:], ones19[:], rinv[:], start=True, stop=True)
                        An = crfpool.tile([T, BC], DTF, tag="A", name="A")
                        nc.vector.tensor_tensor(An[:], A[:], bc_ps[:], mybir.AluOpType.mult)
                        A = An
                        ridx += 1
                # finalize: logZ = ln(sum_j A*exp(end)) + sum_k ln(s_k)
                Afin = crfpool.tile([T, BC], DTF, tag="A", name="A")
                nc.vector.tensor_scalar_mul(Afin[:], A[:], expend[:])
                zb_ps = apool.tile([1, BC], DTF, tag="s", name="s")
                nc.tensor.matmul(zb_ps[:], ones9[:], Afin[:], start=True, stop=True)
                lz = crfpool.tile([1, BC], DTF, tag="ls", name="ls")
                nc.scalar.activation(lz[:], zb_ps[:], mybir.ActivationFunctionType.Ln)
                if ridx > 0:
                    lnS = crfpool.tile([1, BC, NREN], DTF, tag="lnS", name="lnS")
                    nc.scalar.activation(lnS[:], sstore[:, :, 0:ridx],
                                         mybir.ActivationFunctionType.Ln)
                    nc.vector.tensor_reduce(lacc[:], lnS[:], mybir.AxisListType.X,
                                            mybir.AluOpType.add)
                nc.vector.tensor_tensor(logz[:], lz[:], lacc[:], mybir.AluOpType.add)

            nc.sync.dma_start(out_d[0:1, :], numemit[:])
            nc.sync.dma_start(out_d[1:2, :], logz[:])

    nc.compile()
    return nc


# ---------------- host-side preparation ----------------

def _permute_gates(w):
    parts = np.split(np.asarray(w), 4, axis=0)
    return np.concatenate([parts[k] for k in GATE_PERM], axis=0)


def prep_shared(w_ih_f, w_hh_f, b_f, w_ih_b, w_hh_b, b_b, w_proj,
                start_trans, end_trans, transitions):
    out = {}
    for d, (wi, wh, bb) in (("f", (w_ih_f, w_hh_f, b_f)), ("b", (w_ih_b, w_hh_b, b_b))):
        wiP = _permute_gates(wi)  # [4H, E]
        whP = _permute_gates(wh)  # [4H, H]
        bP = _permute_gates(np.asarray(bb)[:, None])[:, 0]
        out[f"wih_{d}"] = np.ascontiguousarray(
            wiP.reshape(MG, 128, KE, 128).transpose(3, 2, 0, 1)
        ).astype(FP8)
        out[f"whh_{d}"] = np.ascontiguousarray(
            whP.reshape(MG, 128, KH, 128).transpose(3, 0, 2, 1)
        ).astype(FP8)
        out[f"bias_{d}"] = bP.reshape(1, MG, 128).astype(FP8)
    out["wproj"] = np.ascontiguousarray(
        np.asarray(w_proj).reshape(T, 4, 128).transpose(2, 1, 0)
    ).astype(BF16)
    out["expM"] = np.exp(np.asarray(transitions, F32))
    out["expst"] = np.exp(np.asarray(start_trans, F32))[:, None]
    out["expend"] = np.exp(np.asarray(end_trans, F32))[:, None]
    return out


def prep_core(emb_shard, tags_shard, b_proj):
    xT = np.ascontiguousarray(
        np.asarray(emb_shard).reshape(BC, S, KE, 128).transpose(3, 2, 1, 0)
    ).astype(BF16)
    oh = np.zeros((T, BC, S), BF16)
    bt = np.arange(BC)[:, None], np.arange(S)[None, :]
    ohf = np.zeros((BC, S, T), np.float32)
    np.put_along_axis(ohf, np.asarray(tags_shard)[..., None], 1.0, axis=-1)
    oh = np.ascontiguousarray(ohf.transpose(2, 0, 1)).astype(BF16)
    return {"xT": xT, "oh": oh, "bproj": np.asarray(b_proj, F32)[:, None]}


def host_path_const(tags, start, end, trans, b_proj):
    tags = np.asarray(tags)
    num = np.asarray(start, F32)[tags[:, 0]]
    num = num + np.asarray(trans, F32)[tags[:, :-1], tags[:, 1:]].sum(axis=1)
    num = num + np.asarray(end, F32)[tags[:, -1]]
    num = num + np.asarray(b_proj, F32)[tags].sum(axis=1)
    return num


_NC_CACHE = {}


def _get_nc(num_devices=N_CORES, s_steps=S, debug=False):
    key = (num_devices, s_steps, debug)
    if key not in _NC_CACHE:
        _NC_CACHE[key] = build_nc(num_devices, s_steps, debug)
    return _NC_CACHE[key]


def kernel(embedding, target_tag, attention_masks, w_ih_f, w_hh_f, b_f,
           w_ih_b, w_hh_b, b_b, w_proj, b_proj, start_trans, end_trans,
           transitions, _debug=False, _trace=False, _tmpdir=None):
    embedding = np.asarray(embedding)
    target_tag = np.asarray(target_tag, np.int32)
    shared = prep_shared(w_ih_f, w_hh_f, b_f, w_ih_b, w_hh_b, b_b, w_proj,
                         start_trans, end_trans, transitions)
    nc = _get_nc(N_CORES, S, _debug)
    in_maps = []
    num_hosts = []
    for i in range(N_CORES):
        sl = slice(i * BC, (i + 1) * BC)
        m = dict(shared)
        m.update(prep_core(embedding[sl], target_tag[sl], b_proj))
        in_maps.append(m)
        num_hosts.append(host_path_const(target_tag[sl], start_trans, end_trans,
                                         transitions, b_proj))
    kw = {}
    if _trace:
        kw = {"trace": True, "tmpdir": _tmpdir}
    res = run_bass_kernel_spmd(nc, in_maps, list(range(N_CORES)), **kw)
    llh = np.zeros((B,), F32)
    for i in range(N_CORES):
        o = res.results[i]["out_nm"]
        llh[i * BC : (i + 1) * BC] = num_hosts[i] + o[0] - o[1]
    out = F32(-llh.mean())
    if _debug or _trace:
        kernel.last_results = res
    return out


# revision 14
# speedup vs baseline: 1.1205x; 1.0111x over previous
"""BiLSTM + CRF loss kernel for Trainium2 (8 NeuronCores, data-parallel over batch).

Problem: nn_BiRNN_CRF — B=64, S=512, E=768, H=256, T=9 tags.
Output: scalar -mean(log-likelihood).

Strategy (per core, Bc=8 examples, both LSTM directions interleaved):
- gate order permuted host-side to (gc, i, f, o): tanh slice / sigmoid slice contiguous
- input projection x@W_ih^T (+bias via ones-row matmul) computed chunk-wise (16
  timesteps) directly into PSUM; the recurrent matmul h@W_hh^T accumulates onto it
  in place (bank-init matmul pre-sets has_written for the whole bank)
- LSTM weights fp8e4, activations bf16 streams, cell state fp32
- layout: gates on partitions [128p, t, 8grp, Bc] so ACT/DVE use all 128 lanes
- emissions em.T = w_proj.T @ [h_f; h_b] into PSUM [9, Bc, S]
- CRF in renormalized linear space: A_t = (expM.T @ A_{t-1}) * exp(em_t + b_proj),
  renorm every 16 steps via ln/exp (factor cancels exactly in logZ)
- numerator: one-hot masked emission sum on device; start/trans/end/b_proj path
  terms computed host-side from int inputs
"""
import sys

sys.path.insert(0, "/opt/trn_rl_repo")

import numpy as np
import ml_dtypes

from concourse import bacc, mybir, tile
from concourse.bass_utils import run_bass_kernel_spmd

BF16 = ml_dtypes.bfloat16
F32 = np.float32

B, S, E, H, T = 64, 512, 768, 256, 9
N_CORES = 8
BC = B // N_CORES  # 8 examples per core
CH = 16  # timesteps per projection chunk
R_RENORM = 32
CRF_C0 = 2.2  # per-step E centering, exp(-C0) folded into E bias; host adds back
GATE_PERM = (2, 0, 1, 3)  # (i,f,gc,o) -> (gc,i,f,o)
KE = E // 128  # 6 K-chunks for input projection
KH = H // 128  # 2 K-chunks for recurrence
MG = 4 * H // 128  # 8 M-tiles of gates
DT8 = mybir.dt.float8e4
DTB = mybir.dt.bfloat16
DTF = mybir.dt.float32
FP8 = np.dtype(mybir.dt.np(DT8))


def build_nc(num_devices=N_CORES, s_steps=S, debug=False):
    """Build the SPMD program (identical on all cores)."""
    SS = s_steps
    NCH = SS // CH
    nc = bacc.Bacc("TRN2", target_bir_lowering=False, debug=False, num_devices=num_devices)

    dp = lambda name, shape, dt: nc.declare_dram_parameter(name, list(shape), dt, isOutput=False)
    # inputs (per core shard)
    xT_d = dp("xT", [128, KE, SS, BC], DTB)  # x transposed [p, k, t, b]
    wih_d = {d: dp(f"wih_{d}", [128, KE, MG, 128], DT8) for d in "fb"}
    whh_d = {d: dp(f"whh_{d}", [128, MG, KH, 128], DT8) for d in "fb"}
    bias_d = {d: dp(f"bias_{d}", [1, MG, 128], DT8) for d in "fb"}
    wproj_d = dp("wproj", [128, 4, T], DTB)
    expM_d = dp("expM", [T, T], DTF)
    expst_d = dp("expst", [T, 1], DTF)
    expend_d = dp("expend", [T, 1], DTF)
    bproj_d = dp("bproj", [T, 1], DTF)
    oh_d = dp("oh", [T, BC, SS], DTB)
    out_d = nc.declare_dram_parameter("out_nm", [2, BC], DTF, isOutput=True)
    if debug:
        hf_dbg = nc.declare_dram_parameter("h_f_dbg", [128, KH, BC, SS], DTB, isOutput=True)
        hb_dbg = nc.declare_dram_parameter("h_b_dbg", [128, KH, BC, SS], DTB, isOutput=True)
        em_dbg = nc.declare_dram_parameter("em_dbg", [T, BC, SS], DTF, isOutput=True)

    with tile.TileContext(nc) as tc:
        with (
            tc.tile_pool(name="const", bufs=1) as cpool,
            tc.tile_pool(name="xchunks", bufs=4) as xpool,
            tc.tile_pool(name="cell", bufs=3) as spool,
            tc.tile_pool(name="crf", bufs=3) as crfpool,
        ):
            # ---- persistent SBUF tiles
            wih = {d: cpool.tile([128, KE, MG, 128], DT8, tag=f"wih{d}", name=f"wih{d}") for d in "fb"}
            whh = {d: cpool.tile([128, MG, KH, 128], DT8, tag=f"whh{d}", name=f"whh{d}") for d in "fb"}
            bias = {d: cpool.tile([1, MG, 128], DT8, tag=f"bias{d}", name=f"bias{d}") for d in "fb"}
            wproj = cpool.tile([128, 4, T], DTB, tag="wproj", name="wproj")
            expM = cpool.tile([T, T], DTF, tag="expM", name="expM")
            expst = cpool.tile([T, 1], DTF, tag="expst", name="expst")
            expend = cpool.tile([T, 1], DTF, tag="expend", name="expend")
            bproj = cpool.tile([T, 1], DTF, tag="bproj", name="bproj")
            oh = cpool.tile([T, BC, SS], DTB, tag="oh", name="oh")
            hst = {d: cpool.tile([128, KH, BC, SS], DTB, tag=f"hst{d}", name=f"hst{d}") for d in "fb"}
            ct = {d: cpool.tile([128, KH, BC], DTF, tag=f"c{d}", name=f"c{d}") for d in "fb"}
            ones_row = cpool.tile([1, 512], DTB, tag="ones_row", name="ones_row")
            zrow = cpool.tile([1, 128], DT8, tag="zrow", name="zrow")
            ones9 = cpool.tile([T, 1], DTF, tag="ones9", name="ones9")
            ones19 = cpool.tile([1, T], DTF, tag="ones19", name="ones19")
            E_sb = cpool.tile([T, BC, SS], DTF, tag="E_sb", name="E_sb")
            lacc = cpool.tile([1, BC], DTF, tag="lacc", name="lacc")
            numemit = cpool.tile([1, BC], DTF, tag="numemit", name="numemit")
            logz = cpool.tile([1, BC], DTF, tag="logz", name="logz")

            for d in "fb":
                nc.sync.dma_start(wih[d][:], wih_d[d][:])
                nc.sync.dma_start(bias[d][:], bias_d[d][:])
                nc.sync.dma_start(whh[d][:], whh_d[d][:])
            nc.vector.memset(ones_row[:], 1.0)
            nc.vector.memset(zrow[:], 0.0)
            nc.vector.memset(ones9[:], 1.0)
            nc.vector.memset(ones19[:], 1.0)
            nc.vector.memset(lacc[:], 0.0)
            for d in "fb":
                nc.vector.memset(ct[d][:], 0.0)

            # ---- phase 1: projection + recurrence
            with tc.tile_pool(name="gpsum", bufs=2, space="PSUM") as gpool:
                xt = {}  # x chunk sbuf tiles per (dir, chunk parity)
                gps = {}  # psum chunk tensors

                def t0_of(d, c):
                    # first global timestep of chunk c's projection slice
                    return c * CH if d == "f" else SS - (c + 1) * CH

                def emit_chunk_dma(d, c):
                    t0 = t0_of(d, c)
                    xtile = xpool.tile([128, KE, CH, BC], DTB, tag=f"x{d}", name=f"x{d}")
                    nc.sync.dma_start(xtile[:], xT_d[:, :, t0 : t0 + CH, :])
                    xt[(d, c)] = xtile

                def proj_thunks(d, c):
                    """Projection of chunk c (dir d) as a list of emission thunks
                    (spread between recurrence steps so they fill PE idle gaps)."""
                    g = gpool.tile([128, MG, CH, BC], DTF, tag=f"g{d}", name=f"g{d}")
                    gps[(d, c)] = g
                    xtile = xt[(d, c)]
                    half = MG // 2
                    thunks = []
                    # k-outer so consecutive matmuls hit different PSUM regions
                    # (same-dst accumulation back-to-back breaks PE pipelining).
                    # start=True only on the first matmul touching each PSUM bank
                    # (clears has_written bank-wide; everything later accumulates)
                    for k in range(KE):
                        for m in range(MG):
                            thunks.append(lambda m=m, k=k: nc.tensor.matmul(
                                g[:, m, :, :],
                                wih[d][:, k, m, :],
                                xtile[:, k, :, :],
                                start=(k == 0 and m % half == 0), stop=False,
                                skip_group_check=True,
                            ))
                    for m in range(MG):
                        thunks.append(lambda m=m: nc.tensor.matmul(
                            g[:, m, :, :],
                            bias[d][:, m, :],
                            ones_row[:, 0 : CH * BC],
                            start=False, stop=False, skip_group_check=True,
                        ))
                    return thunks

                def glob_t(d, c, j):
                    return c * CH + j if d == "f" else SS - 1 - c * CH - j

                def step_mms(d, c, j):
                    t = glob_t(d, c, j)
                    jj = j if d == "f" else CH - 1 - j
                    g = gps[(d, c)]
                    if c == 0 and j == 0:
                        return
                    tprev = t + 1 if d == "b" else t - 1
                    # k-outer: all k=0 matmuls only need h grp0 (written first).
                    # sigmoid gates (m 2..7) first so the sigmoid's deps clear early
                    morder = [2, 3, 4, 5, 6, 7, 0, 1]
                    for k in range(KH):
                        for m in morder:
                            nc.tensor.matmul(
                                g[:, m, jj, :],
                                whh[d][:, m, k, :],
                                hst[d][:, k, :, tprev],
                                start=False,
                                stop=(m == 1 and k == KH - 1),
                                skip_group_check=True,
                            )

                def step_act1(d, c, j):
                    jj = j if d == "f" else CH - 1 - j
                    g = gps[(d, c)]
                    sg = spool.tile([128, 6, BC], DTF, tag=f"sg{d}", name=f"sg{d}")
                    tg = spool.tile([128, 2, BC], DTF, tag=f"tg{d}", name=f"tg{d}")
                    nc.scalar.activation(sg[:], g[:, 2:8, jj, :], mybir.ActivationFunctionType.Sigmoid)
                    nc.scalar.activation(tg[:], g[:, 0:2, jj, :], mybir.ActivationFunctionType.Tanh)
                    return sg, tg

                def step_dve1(d, sg, tg):
                    # cell update (i=sg[0:2], f=sg[2:4], o=sg[4:6])
                    v = spool.tile([128, 2, BC], DTF, tag=f"v{d}", name=f"v{d}")
                    u = spool.tile([128, 2, BC], DTF, tag=f"u{d}", name=f"u{d}")
                    nc.vector.tensor_tensor(v[:], sg[:, 2:4, :], ct[d][:], mybir.AluOpType.mult)
                    nc.vector.tensor_tensor(u[:], sg[:, 0:2, :], tg[:], mybir.AluOpType.mult)
                    nc.vector.tensor_tensor(ct[d][:], u[:], v[:], mybir.AluOpType.add)

                def step_act2(d):
                    th = spool.tile([128, 2, BC], DTF, tag=f"th{d}", name=f"th{d}")
                    nc.scalar.activation(th[:], ct[d][:], mybir.ActivationFunctionType.Tanh)
                    return th

                def step_dve2(d, c, j, sg, th):
                    # split by h-group so grp0 lands first (k=0 matmuls unblock)
                    t = glob_t(d, c, j)
                    for k in range(KH):
                        nc.vector.tensor_tensor(
                            hst[d][:, k, :, t], sg[:, 4 + k, :], th[:, k, :],
                            mybir.AluOpType.mult,
                        )

                def emit_step(d, c, j):
                    # full per-direction sequence: keeps each engine's in-order
                    # queue free of cross-direction stalls
                    step_mms(d, c, j)
                    sg, tg = step_act1(d, c, j)
                    step_dve1(d, sg, tg)
                    th = step_act2(d)
                    step_dve2(d, c, j, sg, th)

                # prologue: chunk 0+1 for both dirs (x DMAs prefetch 2 chunks deep)
                for d in "fb":
                    emit_chunk_dma(d, 0)
                for d in "fb":
                    if NCH > 1:
                        emit_chunk_dma(d, 1)
                    for th_ in proj_thunks(d, 0):
                        th_()
                nc.sync.dma_start(wproj[:], wproj_d[:])
                nc.sync.dma_start(expM[:], expM_d[:])
                nc.sync.dma_start(expst[:], expst_d[:])
                nc.sync.dma_start(expend[:], expend_d[:])
                nc.sync.dma_start(bproj[:], bproj_d[:])
                nc.sync.dma_start(oh[:], oh_d[:])
                for c in range(NCH):
                    thunks = []
                    if c + 1 < NCH:
                        thunks = proj_thunks("f", c + 1) + proj_thunks("b", c + 1)
                    # spread proj over slots 2..CH-1: slot-0/1 thunks would reach the
                    # PE queue head before the psum buffer / x DMA are ready and
                    # stall the in-order queue
                    lo = 2 if CH > 4 else 0
                    per = (len(thunks) + (CH - lo) - 1) // (CH - lo) if thunks else 0
                    for j in range(CH):
                        emit_step("f", c, j)
                        emit_step("b", c, j)
                        if j == 0 and c + 2 < NCH:
                            for d in "fb":
                                emit_chunk_dma(d, c + 2)
                        if thunks and j >= lo:
                            for th_ in thunks[(j - lo) * per : (j - lo + 1) * per]:
                                th_()

            if debug:
                for d, dbg in (("f", hf_dbg), ("b", hb_dbg)):
                    nc.sync.dma_start(dbg[:], hst[d][:])

            # ---- phase 2: emissions + numerator + CRF
            with tc.tile_pool(name="empsum", bufs=1, space="PSUM") as empool:
                em = empool.tile([T, BC, SS], DTF, tag="em", name="em")
                for b in range(BC):
                    for k in range(4):
                        d = "f" if k < 2 else "b"
                        nc.tensor.matmul(
                            em[:, b, :],
                            wproj[:, k, :],
                            hst[d][:, k % 2, b, :],
                            start=(k == 0), stop=(k == 3),
                        )
                if debug:
                    emdbg_sb = crfpool.tile([T, BC, SS], DTF, tag="emdbg", name="emdbg")
                    nc.vector.tensor_copy(emdbg_sb[:], em[:])
                    nc.sync.dma_start(em_dbg[:], emdbg_sb[:])
                # E = exp(em + b_proj)
                nc.scalar.activation(E_sb[:], em[:], mybir.ActivationFunctionType.Exp, bias=bproj[:])
                # numerator: masked emission sum
                msk = crfpool.tile([T, BC, SS], DTF, tag="msk", name="msk")
                nc.vector.tensor_tensor(msk[:], em[:], oh[:], mybir.AluOpType.mult)
                red = crfpool.tile([T, BC], DTF, tag="red", name="red")
                nc.vector.tensor_reduce(red[:], msk[:], mybir.AxisListType.X, mybir.AluOpType.add)

            NREN = (SS - 1) // R_RENORM
            with tc.tile_pool(name="crfpsum", bufs=2, space="PSUM") as apool:
                ne_ps = apool.tile([1, BC], DTF, tag="s", name="s")
                nc.tensor.matmul(ne_ps[:], ones9[:], red[:], start=True, stop=True)
                nc.vector.tensor_copy(numemit[:], ne_ps[:])
                sstore = crfpool.tile([1, BC, max(NREN, 1)], DTF, tag="sstore",
                                      name="sstore", bufs=1)

                # CRF linear-space recursion (no ACT in the loop: renorm via
                # DVE reciprocal, logs of the saved scales taken once at the end)
                A = crfpool.tile([T, BC], DTF, tag="A", name="A")
                nc.vector.tensor_scalar_mul(A[:], E_sb[:, :, 0], expst[:])
                ridx = 0
                for t in range(1, SS):
                    A_ps = apool.tile([T, BC], DTF, tag="Aps", name="Aps")
                    nc.tensor.matmul(A_ps[:], expM[:], A[:], start=True, stop=True)
                    A = crfpool.tile([T, BC], DTF, tag="A", name="A")
                    nc.vector.tensor_tensor(A[:], A_ps[:], E_sb[:, :, t], mybir.AluOpType.mult)
                    if t % R_RENORM == 0:
                        s_ps = apool.tile([1, BC], DTF, tag="s", name="s")
                        nc.tensor.matmul(s_ps[:], ones9[:], A[:], start=True, stop=True)
                        nc.vector.tensor_copy(sstore[:, :, ridx], s_ps[:])
                        rinv = crfpool.tile([1, BC], DTF, tag="rinv", name="rinv")
                        nc.vector.reciprocal(rinv[:], s_ps[:])
                        bc_ps = apool.tile([T, BC], DTF, tag="Aps", name="Aps")
                        nc.tensor.matmul(bc_ps[:], ones19[:], rinv[:], start=True, stop=True)
                        An = crfpool.tile([T, BC], DTF, tag="A", name="A")
                        nc.vector.tensor_tensor(An[:], A[:], bc_ps[:], mybir.AluOpType.mult)
                        A = An
                        ridx += 1
                # finalize: logZ = ln(sum_j A*exp(end)) + sum_k ln(s_k)
                Afin = crfpool.tile([T, BC], DTF, tag="A", name="A")
                nc.vector.tensor_scalar_mul(Afin[:], A[:], expend[:])
                zb_ps = apool.tile([1, BC], DTF, tag="s", name="s")
                nc.tensor.matmul(zb_ps[:], ones9[:], Afin[:], start=True, stop=True)
                lz = crfpool.tile([1, BC], DTF, tag="ls", name="ls")
                nc.scalar.activation(lz[:], zb_ps[:], mybir.ActivationFunctionType.Ln)
                if ridx > 0:
                    lnS = crfpool.tile([1, BC, NREN], DTF, tag="lnS", name="lnS")
                    nc.scalar.activation(lnS[:], sstore[:, :, 0:ridx],
                                         mybir.ActivationFunctionType.Ln)
                    nc.vector.tensor_reduce(lacc[:], lnS[:], mybir.AxisListType.X,
                                            mybir.AluOpType.add)
                nc.vector.tensor_tensor(logz[:], lz[:], lacc[:], mybir.AluOpType.add)

            nc.sync.dma_start(out_d[0:1, :], numemit[:])
            nc.sync.dma_start(out_d[1:2, :], logz[:])

    nc.compile()
    return nc


# ---------------- host-side preparation ----------------

def _permute_gates(w):
    parts = np.split(np.asarray(w), 4, axis=0)
    return np.concatenate([parts[k] for k in GATE_PERM], axis=0)


def prep_shared(w_ih_f, w_hh_f, b_f, w_ih_b, w_hh_b, b_b, w_proj,
                start_trans, end_trans, transitions):
    out = {}
    for d, (wi, wh, bb) in (("f", (w_ih_f, w_hh_f, b_f)), ("b", (w_ih_b, w_hh_b, b_b))):
        wiP = _permute_gates(wi)  # [4H, E]
        whP = _permute_gates(wh)  # [4H, H]
        bP = _permute_gates(np.asarray(bb)[:, None])[:, 0]
        out[f"wih_{d}"] = np.ascontiguousarray(
            wiP.reshape(MG, 128, KE, 128).transpose(3, 2, 0, 1)
        ).astype(FP8)
        out[f"whh_{d}"] = np.ascontiguousarray(
            whP.reshape(MG, 128, KH, 128).transpose(3, 0, 2, 1)
        ).astype(FP8)
        out[f"bias_{d}"] = bP.reshape(1, MG, 128).astype(FP8)
    out["wproj"] = np.ascontiguousarray(
        np.asarray(w_proj).reshape(T, 4, 128).transpose(2, 1, 0)
    ).astype(BF16)
    out["expM"] = np.exp(np.asarray(transitions, F32))
    out["expst"] = np.exp(np.asarray(start_trans, F32))[:, None]
    out["expend"] = np.exp(np.asarray(end_trans, F32))[:, None]
    return out


def prep_core(emb_shard, tags_shard, b_proj):
    xT = np.ascontiguousarray(
        np.asarray(emb_shard).reshape(BC, S, KE, 128).transpose(3, 2, 1, 0)
    ).astype(BF16)
    oh = np.zeros((T, BC, S), BF16)
    bt = np.arange(BC)[:, None], np.arange(S)[None, :]
    ohf = np.zeros((BC, S, T), np.float32)
    np.put_along_axis(ohf, np.asarray(tags_shard)[..., None], 1.0, axis=-1)
    oh = np.ascontiguousarray(ohf.transpose(2, 0, 1)).astype(BF16)
    return {"xT": xT, "oh": oh, "bproj": np.asarray(b_proj, F32)[:, None]}


def host_path_const(tags, start, end, trans, b_proj):
    tags = np.asarray(tags)
    num = np.asarray(start, F32)[tags[:, 0]]
    num = num + np.asarray(trans, F32)[tags[:, :-1], tags[:, 1:]].sum(axis=1)
    num = num + np.asarray(end, F32)[tags[:, -1]]
    num = num + np.asarray(b_proj, F32)[tags].sum(axis=1)
    return num


_NC_CACHE = {}


def _get_nc(num_devices=N_CORES, s_steps=S, debug=False):
    key = (num_devices, s_steps, debug)
    if key not in _NC_CACHE:
        _NC_CACHE[key] = build_nc(num_devices, s_steps, debug)
    return _NC_CACHE[key]


def kernel(embedding, target_tag, attention_masks, w_ih_f, w_hh_f, b_f,
           w_ih_b, w_hh_b, b_b, w_proj, b_proj, start_trans, end_trans,
           transitions, _debug=False, _trace=False, _tmpdir=None):
    embedding = np.asarray(embedding)
    target_tag = np.asarray(target_tag, np.int32)
    shared = prep_shared(w_ih_f, w_hh_f, b_f, w_ih_b, w_hh_b, b_b, w_proj,
                         start_trans, end_trans, transitions)
    nc = _get_nc(N_CORES, S, _debug)
    in_maps = []
    num_hosts = []
    for i in range(N_CORES):
        sl = slice(i * BC, (i + 1) * BC)
        m = dict(shared)
        m.update(prep_core(embedding[sl], target_tag[sl], b_proj))
        m["bproj"] = m["bproj"] - F32(CRF_C0)
        in_maps.append(m)
        num_hosts.append(host_path_const(target_tag[sl], start_trans, end_trans,
                                         transitions, b_proj))
    kw = {}
    if _trace:
        kw = {"trace": True, "tmpdir": _tmpdir}
    res = run_bass_kernel_spmd(nc, in_maps, list(range(N_CORES)), **kw)
    llh = np.zeros((B,), F32)
    for i in range(N_CORES):
        o = res.results[i]["out_nm"]
        llh[i * BC : (i + 1) * BC] = num_hosts[i] + o[0] - (o[1] + S * F32(CRF_C0))
    out = F32(-llh.mean())
    if _debug or _trace:
        kernel.last_results = res
    return out


# revision 16
# speedup vs baseline: 1.1260x; 1.0049x over previous
"""BiLSTM + CRF loss kernel for Trainium2 (8 NeuronCores, data-parallel over batch).

Problem: nn_BiRNN_CRF — B=64, S=512, E=768, H=256, T=9 tags.
Output: scalar -mean(log-likelihood).

Strategy (per core, Bc=8 examples, both LSTM directions interleaved):
- gate order permuted host-side to (gc, i, f, o): tanh slice / sigmoid slice contiguous
- input projection x@W_ih^T (+bias via ones-row matmul) computed chunk-wise (16
  timesteps) directly into PSUM; the recurrent matmul h@W_hh^T accumulates onto it
  in place (bank-init matmul pre-sets has_written for the whole bank)
- LSTM weights fp8e4, activations bf16 streams, cell state fp32
- layout: gates on partitions [128p, t, 8grp, Bc] so ACT/DVE use all 128 lanes
- emissions em.T = w_proj.T @ [h_f; h_b] into PSUM [9, Bc, S]
- CRF in renormalized linear space: A_t = (expM.T @ A_{t-1}) * exp(em_t + b_proj),
  renorm every 16 steps via ln/exp (factor cancels exactly in logZ)
- numerator: one-hot masked emission sum on device; start/trans/end/b_proj path
  terms computed host-side from int inputs
"""
import sys

sys.path.insert(0, "/opt/trn_rl_repo")

import numpy as np
import ml_dtypes

from concourse import bacc, mybir, tile
from concourse.bass_utils import run_bass_kernel_spmd

BF16 = ml_dtypes.bfloat16
F32 = np.float32

B, S, E, H, T = 64, 512, 768, 256, 9
N_CORES = 8
BC = B // N_CORES  # 8 examples per core
CH = 16  # timesteps per projection chunk
R_RENORM = 32
CRF_C0 = 2.2  # per-step E centering, exp(-C0) folded into E bias; host adds back
GATE_PERM = (2, 0, 1, 3)  # (i,f,gc,o) -> (gc,i,f,o)
KE = E // 128  # 6 K-chunks for input projection
KH = H // 128  # 2 K-chunks for recurrence
MG = 4 * H // 128  # 8 M-tiles of gates
DT8 = mybir.dt.float8e4
DTB = mybir.dt.bfloat16
DTF = mybir.dt.float32
FP8 = np.dtype(mybir.dt.np(DT8))


def build_nc(num_devices=N_CORES, s_steps=S, debug=False):
    """Build the SPMD program (identical on all cores)."""
    SS = s_steps
    NCH = SS // CH
    nc = bacc.Bacc("TRN2", target_bir_lowering=False, debug=False, num_devices=num_devices)

    dp = lambda name, shape, dt: nc.declare_dram_parameter(name, list(shape), dt, isOutput=False)
    # inputs (per core shard)
    xT_d = dp("xT", [128, KE, SS, BC], DTB)  # x transposed [p, k, t, b]
    wih_d = {d: dp(f"wih_{d}", [128, KE, MG, 128], DT8) for d in "fb"}
    whh_d = {d: dp(f"whh_{d}", [128, MG, KH, 128], DT8) for d in "fb"}
    bias_d = {d: dp(f"bias_{d}", [1, MG, 128], DT8) for d in "fb"}
    wproj_d = dp("wproj", [128, 4, T], DTB)
    expM_d = dp("expM", [T, T], DTF)
    expst_d = dp("expst", [T, 1], DTF)
    expend_d = dp("expend", [T, 1], DTF)
    bproj_d = dp("bproj", [T, 1], DTF)
    oh_d = dp("oh", [T, BC, SS], DTB)
    out_d = nc.declare_dram_parameter("out_nm", [2, BC], DTF, isOutput=True)
    if debug:
        hf_dbg = nc.declare_dram_parameter("h_f_dbg", [128, KH, BC, SS], DTB, isOutput=True)
        hb_dbg = nc.declare_dram_parameter("h_b_dbg", [128, KH, BC, SS], DTB, isOutput=True)
        em_dbg = nc.declare_dram_parameter("em_dbg", [T, BC, SS], DTF, isOutput=True)

    with tile.TileContext(nc) as tc:
        with (
            tc.tile_pool(name="const", bufs=1) as cpool,
            tc.tile_pool(name="xchunks", bufs=4) as xpool,
            tc.tile_pool(name="cell", bufs=6) as spool,
            tc.tile_pool(name="crf", bufs=3) as crfpool,
        ):
            # ---- persistent SBUF tiles
            wih = {d: cpool.tile([128, KE, MG, 128], DT8, tag=f"wih{d}", name=f"wih{d}") for d in "fb"}
            whh = {d: cpool.tile([128, MG, KH, 128], DT8, tag=f"whh{d}", name=f"whh{d}") for d in "fb"}
            bias = {d: cpool.tile([1, MG, 128], DT8, tag=f"bias{d}", name=f"bias{d}") for d in "fb"}
            wproj = cpool.tile([128, 4, T], DTB, tag="wproj", name="wproj")
            expM = cpool.tile([T, T], DTF, tag="expM", name="expM")
            expst = cpool.tile([T, 1], DTF, tag="expst", name="expst")
            expend = cpool.tile([T, 1], DTF, tag="expend", name="expend")
            bproj = cpool.tile([T, 1], DTF, tag="bproj", name="bproj")
            oh = cpool.tile([T, BC, SS], DTB, tag="oh", name="oh")
            hst = {d: cpool.tile([128, KH, BC, SS], DTB, tag=f"hst{d}", name=f"hst{d}") for d in "fb"}
            ct = {d: cpool.tile([128, KH, BC], DTF, tag=f"c{d}", name=f"c{d}") for d in "fb"}
            ones_row = cpool.tile([1, 512], DTB, tag="ones_row", name="ones_row")
            zrow = cpool.tile([1, 128], DT8, tag="zrow", name="zrow")
            ones9 = cpool.tile([T, 1], DTF, tag="ones9", name="ones9")
            ones19 = cpool.tile([1, T], DTF, tag="ones19", name="ones19")
            E_sb = cpool.tile([T, BC, SS], DTF, tag="E_sb", name="E_sb")
            lacc = cpool.tile([1, BC], DTF, tag="lacc", name="lacc")
            numemit = cpool.tile([1, BC], DTF, tag="numemit", name="numemit")
            logz = cpool.tile([1, BC], DTF, tag="logz", name="logz")

            for d in "fb":
                nc.sync.dma_start(wih[d][:], wih_d[d][:])
                nc.sync.dma_start(bias[d][:], bias_d[d][:])
                nc.sync.dma_start(whh[d][:], whh_d[d][:])
            nc.vector.memset(ones_row[:], 1.0)
            nc.vector.memset(zrow[:], 0.0)
            nc.vector.memset(ones9[:], 1.0)
            nc.vector.memset(ones19[:], 1.0)
            nc.vector.memset(lacc[:], 0.0)
            for d in "fb":
                nc.vector.memset(ct[d][:], 0.0)

            # ---- phase 1: projection + recurrence
            with tc.tile_pool(name="gpsum", bufs=2, space="PSUM") as gpool:
                xt = {}  # x chunk sbuf tiles per (dir, chunk parity)
                gps = {}  # psum chunk tensors

                def t0_of(d, c):
                    # first global timestep of chunk c's projection slice
                    return c * CH if d == "f" else SS - (c + 1) * CH

                def emit_chunk_dma(d, c):
                    t0 = t0_of(d, c)
                    xtile = xpool.tile([128, KE, CH, BC], DTB, tag=f"x{d}", name=f"x{d}")
                    nc.sync.dma_start(xtile[:], xT_d[:, :, t0 : t0 + CH, :])
                    xt[(d, c)] = xtile

                def proj_thunks(d, c):
                    """Projection of chunk c (dir d) as a list of emission thunks
                    (spread between recurrence steps so they fill PE idle gaps)."""
                    g = gpool.tile([128, MG, CH, BC], DTF, tag=f"g{d}", name=f"g{d}")
                    gps[(d, c)] = g
                    xtile = xt[(d, c)]
                    half = MG // 2
                    thunks = []
                    # k-outer so consecutive matmuls hit different PSUM regions
                    # (same-dst accumulation back-to-back breaks PE pipelining).
                    # start=True only on the first matmul touching each PSUM bank
                    # (clears has_written bank-wide; everything later accumulates)
                    for k in range(KE):
                        for m in range(MG):
                            thunks.append(lambda m=m, k=k: nc.tensor.matmul(
                                g[:, m, :, :],
                                wih[d][:, k, m, :],
                                xtile[:, k, :, :],
                                start=(k == 0 and m % half == 0), stop=False,
                                skip_group_check=True,
                            ))
                    for m in range(MG):
                        thunks.append(lambda m=m: nc.tensor.matmul(
                            g[:, m, :, :],
                            bias[d][:, m, :],
                            ones_row[:, 0 : CH * BC],
                            start=False, stop=False, skip_group_check=True,
                        ))
                    return thunks

                def glob_t(d, c, j):
                    return c * CH + j if d == "f" else SS - 1 - c * CH - j

                def step_mms(d, c, j):
                    t = glob_t(d, c, j)
                    jj = j if d == "f" else CH - 1 - j
                    g = gps[(d, c)]
                    if c == 0 and j == 0:
                        return
                    tprev = t + 1 if d == "b" else t - 1
                    # k-outer: all k=0 matmuls only need h grp0 (written first).
                    # sigmoid gates (m 2..7) first so the sigmoid's deps clear early
                    morder = [2, 3, 4, 5, 6, 7, 0, 1]
                    for k in range(KH):
                        for m in morder:
                            nc.tensor.matmul(
                                g[:, m, jj, :],
                                whh[d][:, m, k, :],
                                hst[d][:, k, :, tprev],
                                start=False,
                                stop=(m == 1 and k == KH - 1),
                                skip_group_check=True,
                            )

                def step_act1(d, c, j):
                    jj = j if d == "f" else CH - 1 - j
                    g = gps[(d, c)]
                    sg = spool.tile([128, 6, BC], DTF, tag=f"sg{d}", name=f"sg{d}")
                    tg = spool.tile([128, 2, BC], DTF, tag=f"tg{d}", name=f"tg{d}")
                    nc.scalar.activation(sg[:], g[:, 2:8, jj, :], mybir.ActivationFunctionType.Sigmoid)
                    nc.scalar.activation(tg[:], g[:, 0:2, jj, :], mybir.ActivationFunctionType.Tanh)
                    return sg, tg

                def step_dve1(d, sg, tg):
                    # cell update (i=sg[0:2], f=sg[2:4], o=sg[4:6])
                    v = spool.tile([128, 2, BC], DTF, tag=f"v{d}", name=f"v{d}")
                    u = spool.tile([128, 2, BC], DTF, tag=f"u{d}", name=f"u{d}")
                    nc.vector.tensor_tensor(v[:], sg[:, 2:4, :], ct[d][:], mybir.AluOpType.mult)
                    nc.vector.tensor_tensor(u[:], sg[:, 0:2, :], tg[:], mybir.AluOpType.mult)
                    nc.vector.tensor_tensor(ct[d][:], u[:], v[:], mybir.AluOpType.add)

                def step_act2(d):
                    th = spool.tile([128, 2, BC], DTF, tag=f"th{d}", name=f"th{d}")
                    nc.scalar.activation(th[:], ct[d][:], mybir.ActivationFunctionType.Tanh)
                    return th

                def step_dve2(d, c, j, sg, th):
                    # split by h-group so grp0 lands first (k=0 matmuls unblock)
                    t = glob_t(d, c, j)
                    for k in range(KH):
                        nc.vector.tensor_tensor(
                            hst[d][:, k, :, t], sg[:, 4 + k, :], th[:, k, :],
                            mybir.AluOpType.mult,
                        )

                def emit_step(d, c, j):
                    # full per-direction sequence: keeps each engine's in-order
                    # queue free of cross-direction stalls
                    step_mms(d, c, j)
                    sg, tg = step_act1(d, c, j)
                    step_dve1(d, sg, tg)
                    th = step_act2(d)
                    step_dve2(d, c, j, sg, th)

                # prologue: chunk 0+1 for both dirs (x DMAs prefetch 2 chunks deep)
                for d in "fb":
                    emit_chunk_dma(d, 0)
                for d in "fb":
                    if NCH > 1:
                        emit_chunk_dma(d, 1)
                    for th_ in proj_thunks(d, 0):
                        th_()
                nc.sync.dma_start(wproj[:], wproj_d[:])
                nc.sync.dma_start(expM[:], expM_d[:])
                nc.sync.dma_start(expst[:], expst_d[:])
                nc.sync.dma_start(expend[:], expend_d[:])
                nc.sync.dma_start(bproj[:], bproj_d[:])
                nc.sync.dma_start(oh[:], oh_d[:])
                for c in range(NCH):
                    thunks = []
                    if c + 1 < NCH:
                        thunks = proj_thunks("f", c + 1) + proj_thunks("b", c + 1)
                    # spread proj over slots 2..CH-1: slot-0/1 thunks would reach the
                    # PE queue head before the psum buffer / x DMA are ready and
                    # stall the in-order queue
                    lo = 2 if CH > 4 else 0
                    per = (len(thunks) + (CH - lo) - 1) // (CH - lo) if thunks else 0
                    for j in range(CH):
                        emit_step("f", c, j)
                        emit_step("b", c, j)
                        if j == 0 and c + 2 < NCH:
                            for d in "fb":
                                emit_chunk_dma(d, c + 2)
                        if thunks and j >= lo:
                            for th_ in thunks[(j - lo) * per : (j - lo + 1) * per]:
                                th_()

            if debug:
                for d, dbg in (("f", hf_dbg), ("b", hb_dbg)):
                    nc.sync.dma_start(dbg[:], hst[d][:])

            # ---- phase 2: emissions + numerator + CRF
            with tc.tile_pool(name="empsum", bufs=1, space="PSUM") as empool:
                em = empool.tile([T, BC, SS], DTF, tag="em", name="em")
                red = crfpool.tile([T, BC], DTF, tag="red", name="red", bufs=1)
                msk = crfpool.tile([T, SS], DTF, tag="msk", name="msk")
                # pipelined per-example: PE (em) -> ACT (exp) -> DVE (mask+reduce)
                for b in range(BC):
                    for k in range(4):
                        d = "f" if k < 2 else "b"
                        nc.tensor.matmul(
                            em[:, b, :],
                            wproj[:, k, :],
                            hst[d][:, k % 2, b, :],
                            start=(k == 0), stop=(k == 3),
                        )
                    nc.scalar.activation(E_sb[:, b, :], em[:, b, :],
                                         mybir.ActivationFunctionType.Exp, bias=bproj[:])
                    msk = crfpool.tile([T, SS], DTF, tag="msk", name="msk")
                    nc.vector.tensor_tensor(msk[:], em[:, b, :], oh[:, b, :],
                                            mybir.AluOpType.mult)
                    nc.vector.tensor_reduce(red[:, b : b + 1], msk[:],
                                            mybir.AxisListType.X, mybir.AluOpType.add)
                if debug:
                    emdbg_sb = crfpool.tile([T, BC, SS], DTF, tag="emdbg", name="emdbg")
                    nc.vector.tensor_copy(emdbg_sb[:], em[:])
                    nc.sync.dma_start(em_dbg[:], emdbg_sb[:])

            NREN = (SS - 1) // R_RENORM
            with tc.tile_pool(name="crfpsum", bufs=2, space="PSUM") as apool:
                ne_ps = apool.tile([1, BC], DTF, tag="s", name="s")
                nc.tensor.matmul(ne_ps[:], ones9[:], red[:], start=True, stop=True)
                nc.vector.tensor_copy(numemit[:], ne_ps[:])
                sstore = crfpool.tile([1, BC, max(NREN, 1)], DTF, tag="sstore",
                                      name="sstore", bufs=1)

                # CRF linear-space recursion (no ACT in the loop: renorm via
                # DVE reciprocal, logs of the saved scales taken once at the end)
                A = crfpool.tile([T, BC], DTF, tag="A", name="A")
                nc.vector.tensor_scalar_mul(A[:], E_sb[:, :, 0], expst[:])
                ridx = 0
                for t in range(1, SS):
                    A_ps = apool.tile([T, BC], DTF, tag="Aps", name="Aps")
                    nc.tensor.matmul(A_ps[:], expM[:], A[:], start=True, stop=True)
                    A = crfpool.tile([T, BC], DTF, tag="A", name="A")
                    nc.vector.tensor_tensor(A[:], A_ps[:], E_sb[:, :, t], mybir.AluOpType.mult)
                    if t % R_RENORM == 0:
                        s_ps = apool.tile([1, BC], DTF, tag="s", name="s")
                        nc.tensor.matmul(s_ps[:], ones9[:], A[:], start=True, stop=True)
                        nc.vector.tensor_copy(sstore[:, :, ridx], s_ps[:])
                        rinv = crfpool.tile([1, BC], DTF, tag="rinv", name="rinv")
                        nc.vector.reciprocal(rinv[:], s_ps[:])
                        bc_ps = apool.tile([T, BC], DTF, tag="Aps", name="Aps")
                        nc.tensor.matmul(bc_ps[:], ones19[:], rinv[:], start=True, stop=True)
                        An = crfpool.tile([T, BC], DTF, tag="A", name="A")
                        nc.vector.tensor_tensor(An[:], A[:], bc_ps[:], mybir.AluOpType.mult)
                        A = An
                        ridx += 1
                # finalize: logZ = ln(sum_j A*exp(end)) + sum_k ln(s_k)
                Afin = crfpool.tile([T, BC], DTF, tag="A", name="A")
                nc.vector.tensor_scalar_mul(Afin[:], A[:], expend[:])
                zb_ps = apool.tile([1, BC], DTF, tag="s", name="s")
                nc.tensor.matmul(zb_ps[:], ones9[:], Afin[:], start=True, stop=True)
                lz = crfpool.tile([1, BC], DTF, tag="ls", name="ls")
                nc.scalar.activation(lz[:], zb_ps[:], mybir.ActivationFunctionType.Ln)
                if ridx > 0:
                    lnS = crfpool.tile([1, BC, NREN], DTF, tag="lnS", name="lnS")
                    nc.scalar.activation(lnS[:], sstore[:, :, 0:ridx],
                                         mybir.ActivationFunctionType.Ln)
                    nc.vector.tensor_reduce(lacc[:], lnS[:], mybir.AxisListType.X,
                                            mybir.AluOpType.add)
                nc.vector.tensor_tensor(logz[:], lz[:], lacc[:], mybir.AluOpType.add)

            nc.sync.dma_start(out_d[0:1, :], numemit[:])
            nc.sync.dma_start(out_d[1:2, :], logz[:])

    nc.compile()
    return nc


# ---------------- host-side preparation ----------------

def _permute_gates(w):
    parts = np.split(np.asarray(w), 4, axis=0)
    return np.concatenate([parts[k] for k in GATE_PERM], axis=0)


def prep_shared(w_ih_f, w_hh_f, b_f, w_ih_b, w_hh_b, b_b, w_proj,
                start_trans, end_trans, transitions):
    out = {}
    for d, (wi, wh, bb) in (("f", (w_ih_f, w_hh_f, b_f)), ("b", (w_ih_b, w_hh_b, b_b))):
        wiP = _permute_gates(wi)  # [4H, E]
        whP = _permute_gates(wh)  # [4H, H]
        bP = _permute_gates(np.asarray(bb)[:, None])[:, 0]
        out[f"wih_{d}"] = np.ascontiguousarray(
            wiP.reshape(MG, 128, KE, 128).transpose(3, 2, 0, 1)
        ).astype(FP8)
        out[f"whh_{d}"] = np.ascontiguousarray(
            whP.reshape(MG, 128, KH, 128).transpose(3, 0, 2, 1)
        ).astype(FP8)
        out[f"bias_{d}"] = bP.reshape(1, MG, 128).astype(FP8)
    out["wproj"] = np.ascontiguousarray(
        np.asarray(w_proj).reshape(T, 4, 128).transpose(2, 1, 0)
    ).astype(BF16)
    out["expM"] = np.exp(np.asarray(transitions, F32))
    out["expst"] = np.exp(np.asarray(start_trans, F32))[:, None]
    out["expend"] = np.exp(np.asarray(end_trans, F32))[:, None]
    return out


def prep_core(emb_shard, tags_shard, b_proj):
    xT = np.ascontiguousarray(
        np.asarray(emb_shard).reshape(BC, S, KE, 128).transpose(3, 2, 1, 0)
    ).astype(BF16)
    oh = np.zeros((T, BC, S), BF16)
    bt = np.arange(BC)[:, None], np.arange(S)[None, :]
    ohf = np.zeros((BC, S, T), np.float32)
    np.put_along_axis(ohf, np.asarray(tags_shard)[..., None], 1.0, axis=-1)
    oh = np.ascontiguousarray(ohf.transpose(2, 0, 1)).astype(BF16)
    return {"xT": xT, "oh": oh, "bproj": np.asarray(b_proj, F32)[:, None]}


def host_path_const(tags, start, end, trans, b_proj):
    tags = np.asarray(tags)
    num = np.asarray(start, F32)[tags[:, 0]]
    num = num + np.asarray(trans, F32)[tags[:, :-1], tags[:, 1:]].sum(axis=1)
    num = num + np.asarray(end, F32)[tags[:, -1]]
    num = num + np.asarray(b_proj, F32)[tags].sum(axis=1)
    return num


_NC_CACHE = {}


def _get_nc(num_devices=N_CORES, s_steps=S, debug=False):
    key = (num_devices, s_steps, debug)
    if key not in _NC_CACHE:
        _NC_CACHE[key] = build_nc(num_devices, s_steps, debug)
    return _NC_CACHE[key]


def kernel(embedding, target_tag, attention_masks, w_ih_f, w_hh_f, b_f,
           w_ih_b, w_hh_b, b_b, w_proj, b_proj, start_trans, end_trans,
           transitions, _debug=False, _trace=False, _tmpdir=None):
    embedding = np.asarray(embedding)
    target_tag = np.asarray(target_tag, np.int32)
    shared = prep_shared(w_ih_f, w_hh_f, b_f, w_ih_b, w_hh_b, b_b, w_proj,
                         start_trans, end_trans, transitions)
    nc = _get_nc(N_CORES, S, _debug)
    in_maps = []
    num_hosts = []
    for i in range(N_CORES):
        sl = slice(i * BC, (i + 1) * BC)
        m = dict(shared)
        m.update(prep_core(embedding[sl], target_tag[sl], b_proj))
        m["bproj"] = m["bproj"] - F32(CRF_C0)
        in_maps.append(m)
        num_hosts.append(host_path_const(target_tag[sl], start_trans, end_trans,
                                         transitions, b_proj))
    kw = {}
    if _trace:
        kw = {"trace": True, "tmpdir": _tmpdir}
    res = run_bass_kernel_spmd(nc, in_maps, list(range(N_CORES)), **kw)
    llh = np.zeros((B,), F32)
    for i in range(N_CORES):
        o = res.results[i]["out_nm"]
        llh[i * BC : (i + 1) * BC] = num_hosts[i] + o[0] - (o[1] + S * F32(CRF_C0))
    out = F32(-llh.mean())
    if _debug or _trace:
        kernel.last_results = res
    return out
